# revision 9
# baseline (speedup 1.0000x reference)
"""LoRA layer kernel for Trainium2 (8 NeuronCores, data-parallel).

Computes out = SCALING * (x @ A^T) @ B^T for x [4, 8192, 1024],
lora_A [4, 1024], lora_B [1024, 4], SCALING = 0.25.

Strategy (per core, shard = 4096 rows x 1024 features), fp16 end-to-end
(rel err ~6e-4 vs the 2e-2 gate; halves HBM traffic vs f32):
  - Host casts x to fp16 and pre-transposes each core's shard so feature
    chunks sit on partitions -> no PE transposes on device.
  - PE array packing (tile_position): both LoRA matmuls use a tiny slice
    of the 128x128 array (stage 1: M=4 of 128 cols; stage 2: K=4 of 128
    rows), so up to 4 row-groups are processed CONCURRENTLY in disjoint
    32-wide array tiles: stage 1 column-tiled (128x32, ht of group q at
    PSUM partitions 32q..32q+3 of one shared bank), stage 2 row-tiled
    (32x128, lhsT/rhs read SBUF partitions 32q.., each tile into its own
    PSUM bank).
  - Software pipelining: stage-2 of super-group k-1 (per-512-row-group
    packets: 8 matmuls -> 8 evacuations -> 1 MiB store) is emitted
    between the load-gated stage-1 chunk rounds of super-group k, so
    stores flow at DMA pace all through the kernel instead of bunching.
  - Tapered plan [512, 2048, 1024, 512] rows: early first store, short
    final drain. Loads chunk-granular on the sync HWDGE ring; stores on
    the scalar HWDGE ring; ScalarE/DVE alternate PSUM->SBUF evacuation.
  - Host upcasts the fp16 result to f32 and un-permutes.
"""

import sys

for _p in (
    "/root/.axon_site",
    "/root/.axon_site/_ro/trn_rl_repo",
    "/root/.axon_site/_ro/pypackages",
):
    if _p not in sys.path:
        sys.path.insert(0, _p)

from contextlib import ExitStack

import numpy as np

N_CORES = 8
D_IN = 1024
D_OUT = 1024
RANK = 4
ROWS_TOTAL = 4 * 8192
ROWS_PER_CORE = ROWS_TOTAL // N_CORES  # 4096
SCALING = 1.0 / RANK

P = 128               # partitions
N_CHUNKS = D_IN // P  # 8 feature chunks of 128
G = 512               # rows per group (one PSUM accumulation chain)
J = G // P            # 4 row subtiles per group

# Super-group plan: (row0, n_groups). Each sg runs n_groups 512-row groups
# concurrently in disjoint PE array tiles (partition stride 128//n_groups).
PLAN = [(0, 1), (512, 4), (2560, 2), (3584, 1)]
N_JSLICES = ROWS_PER_CORE // P             # 32 output j-slices [128, 1024]
N_XBLOCKS = ROWS_PER_CORE // G * N_CHUNKS  # 64 x-blocks of [128, 512]


def emit_lora(tc, xt_ap, at_ap, bt_ap, out_ap):
    """Emit the LoRA kernel IR for one core's 4096-row shard.

    xt_ap : DRAM [P, 64, 512] fp16; block b = sg-major [c][g][m]:
            xt[p, (row0//512)*8 + c*n_g + g, m] = x[row0 + g*512 + m, c*128 + p]
    at_ap : DRAM [P, N_CHUNKS, RANK] fp16, at[p, c, r] = A[r, c*128 + p]
    bt_ap : DRAM [P, D_OUT] fp16, bt[32k + r, o] = SCALING * B[o, r] (k<4),
            zero elsewhere (replicated for row-tiled stage 2)
    out_ap: DRAM [P, 32, 1024] fp16, out[p, row0//128 + g*4 + j, o] =
            result[row0 + g*512 + j*128 + p, o]
    """
    import concourse.mybir as mybir

    nc = tc.nc
    f32 = mybir.dt.float32
    f16 = mybir.dt.float16
    ctx = tc._ctx  # ExitStack owned by caller

    consts = ctx.enter_context(tc.tile_pool(name="consts", bufs=1))
    xpool = ctx.enter_context(tc.tile_pool(name="xin", bufs=2))
    htpool = ctx.enter_context(tc.tile_pool(name="ht", bufs=3))
    opool = ctx.enter_context(tc.tile_pool(name="osb", bufs=6))
    ps_ht = ctx.enter_context(tc.tile_pool(name="ps_ht", bufs=2, space="PSUM"))
    ps_o = ctx.enter_context(tc.tile_pool(name="ps_o", bufs=5, space="PSUM"))

    # Tiny constants lead the scalar (ACT) HWDGE ring, which is otherwise
    # idle at kernel start; the x loads own the sync ring from t=0.
    at_sb = consts.tile([P, N_CHUNKS, RANK], f16)
    nc.scalar.dma_start(at_sb[:], at_ap[:])
    bt_sb = consts.tile([P, D_OUT], f16)
    nc.scalar.dma_start(bt_sb[:], bt_ap[:])

    evac_ctr = [0]

    def s2_units(row0, n_g, stride, ht_sb):
        """Stage-2 packets, one per 512-row group: 8 row-tiled matmuls,
        8 alternating evacuations, then that group's 1 MiB store."""
        jb = row0 // P
        for q in range(n_g):
            o_sb = opool.tile([P, J, D_OUT], f16, name="o_sb")
            for j in range(J):
                for o2 in range(2):
                    o_ps = ps_o.tile(
                        [P, 512], f32, name="o_ps"
                    )
                    nc.tensor.matmul(
                        o_ps[:],
                        lhsT=ht_sb[
                            q * stride : q * stride + RANK, j * P : (j + 1) * P
                        ],
                        rhs=bt_sb[
                            q * stride : q * stride + RANK,
                            o2 * 512 : (o2 + 1) * 512,
                        ],
                        start=True,
                        stop=True,
                        tile_position=(q * stride, 0),
                    )
                    tgt = o_sb[:, j, o2 * 512 : (o2 + 1) * 512]
                    if evac_ctr[0] % 2 == 0:
                        nc.scalar.copy(tgt, o_ps[:])
                    else:
                        nc.vector.tensor_copy(tgt, o_ps[:])
                    evac_ctr[0] += 1
            # Stores ride the second HWDGE ring (ACT), never head-of-line
            # blocking the sync ring carrying the loads.
            nc.scalar.dma_start(
                out_ap[:, jb + q * J : jb + (q + 1) * J, :], o_sb[:]
            )
            yield

    prev_gen = None
    prev_units = 0
    for row0, n_g in PLAN:
        stride = P // n_g
        xb = row0 // G * N_CHUNKS  # first x-block of this sg

        # Chunk-granular loads: stage-1 round c only needs chunk c of all
        # n_g groups, so compute starts after the first block lands.
        x_sb = xpool.tile([P, N_CHUNKS, n_g, G], f16, name="x_sb")
        for c in range(N_CHUNKS):
            nc.sync.dma_start(
                x_sb[:, c], xt_ap[:, xb + c * n_g : xb + (c + 1) * n_g, :]
            )

        # Stage 1, column-tiled 128x32: group q's chain accumulates into
        # PSUM partitions q*stride..+3 of one shared bank. Round-robin over
        # q inside each chunk round so the n_g array tiles run concurrently.
        # Between rounds, drain stage-2 packets of the previous sg.
        ht_ps = ps_ht.tile([P, G], f32, name="ht_ps")
        done = 0
        for c in range(N_CHUNKS):
            for q in range(n_g):
                nc.tensor.matmul(
                    ht_ps[q * stride : q * stride + RANK, :],
                    lhsT=at_sb[:, c, :],
                    rhs=x_sb[:, c, q, :],
                    start=(c == 0),
                    stop=(c == N_CHUNKS - 1),
                    tile_position=(0, q * stride),
                    skip_group_check=True,
                )
            if prev_gen is not None:
                target = (c + 1) * prev_units // N_CHUNKS
                while done < target:
                    next(prev_gen)
                    done += 1
        if prev_gen is not None:
            for _ in prev_gen:
                pass

        ht_sb = htpool.tile([P, G], f16, name="ht_sb")
        nc.vector.tensor_copy(ht_sb[:], ht_ps[:])
        prev_gen = s2_units(row0, n_g, stride, ht_sb)
        prev_units = n_g

    for _ in prev_gen:
        pass


def build_nc():
    import concourse.mybir as mybir
    import concourse.tile as tile
    from concourse import bacc

    f16 = mybir.dt.float16
    nc = bacc.Bacc("TRN2", target_bir_lowering=False, debug=False)
    xt_d = nc.dram_tensor("xt", [P, N_XBLOCKS, G], f16, kind="ExternalInput").ap()
    at_d = nc.dram_tensor("at", [P, N_CHUNKS, RANK], f16, kind="ExternalInput").ap()
    bt_d = nc.dram_tensor("bt", [P, D_OUT], f16, kind="ExternalInput").ap()
    out_d = nc.dram_tensor(
        "out", [P, N_JSLICES, D_OUT], f16, kind="ExternalOutput"
    ).ap()

    with tile.TileContext(nc) as tc:
        with ExitStack() as ctx:
            tc._ctx = ctx
            emit_lora(tc, xt_d, at_d, bt_d, out_d)
    nc.compile()
    return nc


def host_prep(lora_A, lora_B):
    # at[p, c, r] = A[r, c*128 + p]
    at = np.ascontiguousarray(
        np.asarray(lora_A, dtype=np.float32)
        .reshape(RANK, N_CHUNKS, P)
        .transpose(2, 1, 0)
    ).astype(np.float16)
    # bt[32k + r, o] = SCALING * B[o, r], replicated at partition stride 32
    btv = (np.asarray(lora_B, dtype=np.float32).T * SCALING).astype(np.float16)
    bt = np.zeros((P, D_OUT), dtype=np.float16)
    for k in range(4):
        bt[32 * k : 32 * k + RANK] = btv
    return at, bt


def stage_x(x):
    """x [4, 8192, 1024] f32 -> per-core [P, N_XBLOCKS, 512] fp16 shards."""
    xc = np.asarray(x, dtype=np.float32).reshape(N_CORES, ROWS_PER_CORE, D_IN)
    blocks = []
    for row0, n_g in PLAN:
        rows = n_g * G
        xh = xc[:, row0 : row0 + rows].reshape(N_CORES, n_g, G, N_CHUNKS, P)
        # (core, g, m, c, p) -> (core, p, c, g, m)
        blocks.append(xh.transpose(0, 4, 3, 1, 2).reshape(N_CORES, P, -1))
    xs = np.concatenate(blocks, axis=2).astype(np.float16)
    return np.ascontiguousarray(xs.reshape(N_CORES, P, N_XBLOCKS, G))


def unstage_out(res_list):
    """Per-core [P, N_JSLICES, 1024] fp16 -> out [4, 8192, 1024] f32."""
    o = np.stack(res_list, axis=0).astype(np.float32)
    out = np.empty((N_CORES, ROWS_PER_CORE, D_OUT), dtype=np.float32)
    for row0, n_g in PLAN:
        jb = row0 // P
        blk = o[:, :, jb : jb + n_g * J, :].reshape(N_CORES, P, n_g, J, D_OUT)
        # (core, p, g, j, o) -> (core, g, j, p, o)
        out[:, row0 : row0 + n_g * G] = blk.transpose(0, 2, 3, 1, 4).reshape(
            N_CORES, n_g * G, D_OUT
        )
    return np.ascontiguousarray(out).reshape(4, 8192, D_OUT)


_NC_CACHE = {}


def kernel(x, lora_A, lora_B):
    from concourse.bass_utils import run_bass_kernel_spmd

    if "nc" not in _NC_CACHE:
        _NC_CACHE["nc"] = build_nc()
    nc = _NC_CACHE["nc"]

    xs = stage_x(x)
    at, bt = host_prep(lora_A, lora_B)
    in_maps = [
        {"xt": np.ascontiguousarray(xs[i]), "at": at, "bt": bt}
        for i in range(N_CORES)
    ]
    res = run_bass_kernel_spmd(nc, in_maps, core_ids=list(range(N_CORES)))
    return unstage_out([res.results[i]["out"] for i in range(N_CORES)])
